# revision 1
# baseline (speedup 1.0000x reference)
"""Cross-attention with relative-position-bias MLP on 8 Trainium2 NeuronCores.

Sharding: batch-parallel attention (core c owns batch element c) +
Lq-sharded bias MLP (core c computes bias rows for queries 64c..64c+64),
AllGather of the [512, 12, 512] bias tensor, then full attention per core.

Precision strategy (PE fp32 matmul is 4-8x slower than 16-bit / f32r):
- bias MLP mm1: bf16 hi/lo split packed into K=128 (exact to ~2^-17)
- bias MLP mm2: fp16 hidden x (W2hi + W2lo fp16 split, accumulated in PSUM)
- projections / QK / AV / O: f32r (TF32-class, ~1.5e-4) via AP bitcast
- softmax: fp32 exp with fused row-sum, fp32 transposes

Self-contained: hardcodes all shapes; builds/compiles the Bass kernel on
first call and runs it via bass_utils.run_bass_kernel_spmd on cores 0-7.
"""

import numpy as np

import concourse.bass as bass
import concourse.mybir as mybir
import concourse.tile as tile
from concourse import bacc, bass_utils
from concourse.masks import make_identity

F32 = mybir.dt.float32
F32R = mybir.dt.float32r
BF16 = mybir.dt.bfloat16
FP16 = mybir.dt.float16
AF = mybir.ActivationFunctionType
ADD = mybir.AluOpType.add

NCORES = 8
B = 8
L = 512
D = 768
H = 12
DH = 64
QS = L // NCORES
NCH = D // 128
SCALE = DH ** -0.5

_CACHE = {}


def _build(dbg=False):
    nc = bacc.Bacc("TRN2", target_bir_lowering=False, debug=False, num_devices=NCORES)

    xqT_d = nc.dram_tensor("xqT", [D, L], F32R, kind="ExternalInput")
    kvT_d = nc.dram_tensor("kvT", [D, L], F32R, kind="ExternalInput")
    relP_d = nc.dram_tensor("relP", [128, QS * L], BF16, kind="ExternalInput")
    WqS_d = nc.dram_tensor("WqS", [128, NCH, D], F32R, kind="ExternalInput")
    Wk_d = nc.dram_tensor("Wk", [128, NCH, D], F32R, kind="ExternalInput")
    Wv_d = nc.dram_tensor("Wv", [128, NCH, D], F32R, kind="ExternalInput")
    Wo_d = nc.dram_tensor("Wo", [DH, H, D], F32R, kind="ExternalInput")
    W1P_d = nc.dram_tensor("W1P", [128, D], BF16, kind="ExternalInput")
    W2P_d = nc.dram_tensor("W2P", [128, NCH, 2 * H], FP16, kind="ExternalInput")  # hi|lo
    bqS_d = nc.dram_tensor("bqS", [128, NCH], F32, kind="ExternalInput")
    bk_d = nc.dram_tensor("bk", [128, NCH], F32, kind="ExternalInput")
    b1_d = nc.dram_tensor("b1", [128, NCH], F32, kind="ExternalInput")
    b2_d = nc.dram_tensor("b2", [H, 1], F32, kind="ExternalInput")
    bv_d = nc.dram_tensor("bvb", [128, D], F32, kind="ExternalInput")
    bo_d = nc.dram_tensor("bob", [128, D], F32, kind="ExternalInput")
    out_d = nc.dram_tensor("out", [L, D], F32, kind="ExternalOutput")
    if dbg:
        dbg_bfull = nc.dram_tensor("dbg_bfull", [L * H, L], F32, kind="ExternalOutput")

    with tile.TileContext(nc) as tc:
        with (
            tc.tile_pool(name="dram", bufs=1, space="DRAM") as dpool,
            tc.tile_pool(name="persist", bufs=1) as pp,
        ):
            QH = QS // 2
            bias_shard1 = dpool.tile([QH * H, L], F32, name="bias_shard1")
            bias_shard2 = dpool.tile([QH * H, L], F32, name="bias_shard2")
            bias_full1 = dpool.tile(
                [NCORES * QH * H, L], F32, name="bias_full1", addr_space="Shared"
            )
            bias_full2 = dpool.tile(
                [NCORES * QH * H, L], F32, name="bias_full2", addr_space="Shared"
            )

            W1p_sb = pp.tile([128, D], BF16, name="W1p_sb")
            nc.sync.dma_start(W1p_sb[:], W1P_d[:, :])
            W2P_sb = pp.tile([128, NCH, 2 * H], FP16, name="W2P_sb")
            nc.sync.dma_start(W2P_sb[:], W2P_d[:, :, :])
            Wo_sb = pp.tile([DH, H, D], F32R, name="Wo_sb")
            nc.sync.dma_start(Wo_sb[:], Wo_d[:, :, :])
            b1_sb = pp.tile([128, NCH], F32, name="b1_sb")
            nc.sync.dma_start(b1_sb[:], b1_d[:, :])
            b2_sb = pp.tile([H, 1], F32, name="b2_sb")
            nc.sync.dma_start(b2_sb[:], b2_d[:, :])
            bq_sb = pp.tile([128, NCH], F32, name="bq_sb")
            nc.sync.dma_start(bq_sb[:], bqS_d[:, :])
            bk_sb = pp.tile([128, NCH], F32, name="bk_sb")
            nc.sync.dma_start(bk_sb[:], bk_d[:, :])
            bv_sb = pp.tile([128, D], F32, name="bv_sb")
            nc.sync.dma_start(bv_sb[:], bv_d[:, :])
            bo_sb = pp.tile([128, D], F32, name="bo_sb")
            nc.sync.dma_start(bo_sb[:], bo_d[:, :])
            ident = pp.tile([128, 128], F32, name="ident")
            make_identity(nc, ident[:])

            qT_sb = pp.tile([128, NCH, L], F32R, name="qT_sb")
            kT_sb = pp.tile([128, NCH, L], F32R, name="kT_sb")
            v_sb = pp.tile([128, 4, D], F32R, name="v_sb")
            attnT = pp.tile([DH, H, L], F32R, name="attnT")

            # ---- Phase 1: bias MLP over this core's 64 queries (2q per step) ----
            with (
                tc.tile_pool(name="p1rel", bufs=3) as p1rel,
                tc.tile_pool(name="p1gel", bufs=3) as p1gel,
                tc.tile_pool(name="p1out", bufs=3) as p1out,
                tc.tile_pool(name="p1ps", bufs=2, space="PSUM") as p1ps,
                tc.tile_pool(name="p1psb", bufs=3, space="PSUM") as p1psb,
            ):
                for qq in range(QS // 2):
                    rel2 = p1rel.tile([128, 2 * L], BF16, tag="rel", name=f"rel_{qq}")
                    nc.sync.dma_start(
                        rel2[:], relP_d[:, qq * 2 * L : (qq + 1) * 2 * L]
                    )
                    bps = [
                        p1psb.tile([H, L], F32, tag="bps", name=f"bps_{qq}_{j}")
                        for j in range(2)
                    ]
                    for dc in range(NCH):
                        hidw = p1ps.tile(
                            [128, 2 * L], F32, tag="hid", name=f"hid_{qq}_{dc}"
                        )
                        for j in range(2):
                            nc.tensor.matmul(
                                hidw[:, j * L : (j + 1) * L],
                                W1p_sb[:, dc * 128 : (dc + 1) * 128],
                                rel2[:, j * L : (j + 1) * L],
                                start=True,
                                stop=True,
                            )
                        gelw = p1gel.tile(
                            [128, 2 * L], FP16, tag="gel", name=f"gel_{qq}_{dc}"
                        )
                        nc.scalar.activation(
                            gelw[:], hidw[:], AF.Gelu, bias=b1_sb[:, dc : dc + 1]
                        )
                        for j in range(2):
                            nc.tensor.matmul(
                                bps[j][:],
                                W2P_sb[:, dc, 0:H],
                                gelw[:, j * L : (j + 1) * L],
                                start=(dc == 0),
                                stop=False,
                            )
                            nc.tensor.matmul(
                                bps[j][:],
                                W2P_sb[:, dc, H : 2 * H],
                                gelw[:, j * L : (j + 1) * L],
                                start=False,
                                stop=(dc == NCH - 1),
                            )
                    for j in range(2):
                        q = qq * 2 + j
                        bsb = p1out.tile([H, L], F32, tag="bsb", name=f"bsb_{q}")
                        nc.vector.tensor_scalar_add(bsb[:], bps[j][:], b2_sb[:, 0:1])
                        shard = bias_shard1 if q < QH else bias_shard2
                        qr = q if q < QH else q - QH
                        nc.sync.dma_start(shard[qr * H : (qr + 1) * H, :], bsb[:])
                    if qq == QS // 4 - 1:
                        nc.gpsimd.collective_compute(
                            "AllGather",
                            mybir.AluOpType.bypass,
                            replica_groups=[list(range(NCORES))],
                            ins=[bias_shard1[:].opt()],
                            outs=[bias_full1[:].opt()],
                        )

            nc.gpsimd.collective_compute(
                "AllGather",
                mybir.AluOpType.bypass,
                replica_groups=[list(range(NCORES))],
                ins=[bias_shard2[:].opt()],
                outs=[bias_full2[:].opt()],
            )

            # ---- Phase 3a: q/k/v projections (f32r, overlaps the all-gather) ----
            with (
                tc.tile_pool(name="wpool", bufs=1) as wp,
                tc.tile_pool(name="ptmp", bufs=3) as ptmp,
                tc.tile_pool(name="pps", bufs=2, space="PSUM") as pps,
            ):
                WqS_sb = wp.tile([128, NCH, D], F32R, name="WqS_sb")
                nc.sync.dma_start(WqS_sb[:], WqS_d[:, :, :])
                Wk_sb = wp.tile([128, NCH, D], F32R, name="Wk_sb")
                nc.sync.dma_start(Wk_sb[:], Wk_d[:, :, :])
                Wv_sb = wp.tile([128, NCH, D], F32R, name="Wv_sb")
                nc.sync.dma_start(Wv_sb[:], Wv_d[:, :, :])
                xqT_sb = wp.tile([128, NCH, L], F32R, name="xqT_sb")
                nc.sync.dma_start(
                    xqT_sb[:], xqT_d.ap().rearrange("(c p) t -> p c t", p=128)
                )
                kvT_sb = wp.tile([128, NCH, L], F32R, name="kvT_sb")
                nc.sync.dma_start(
                    kvT_sb[:], kvT_d.ap().rearrange("(c p) t -> p c t", p=128)
                )

                def proj(W_sb, x_sb, b_sb, out_t, pfx):
                    for oc in range(NCH):
                        ps = pps.tile([128, L], F32, tag="psp", name=f"pp{pfx}_{oc}")
                        for di in range(NCH):
                            nc.tensor.matmul(
                                ps[:],
                                W_sb[:, di, oc * 128 : (oc + 1) * 128],
                                x_sb[:, di, :],
                                start=(di == 0),
                                stop=(di == NCH - 1),
                            )
                        nc.vector.tensor_scalar_add(
                            out_t[:, oc, :], ps[:], b_sb[:, oc : oc + 1]
                        )

                proj(WqS_sb, xqT_sb, bq_sb, qT_sb, "q")
                proj(Wk_sb, kvT_sb, bk_sb, kT_sb, "k")
                for tc4 in range(4):
                    for hf in range(2):
                        ps = pps.tile([128, 384], F32, tag="psv", name=f"ppv_{tc4}_{hf}")
                        for di in range(NCH):
                            nc.tensor.matmul(
                                ps[:],
                                kvT_sb[:, di, tc4 * 128 : (tc4 + 1) * 128],
                                Wv_sb[:, di, hf * 384 : (hf + 1) * 384],
                                start=(di == 0),
                                stop=(di == NCH - 1),
                            )
                        nc.vector.tensor_tensor(
                            v_sb[:, tc4, hf * 384 : (hf + 1) * 384],
                            ps[:],
                            bv_sb[:, hf * 384 : (hf + 1) * 384],
                            op=ADD,
                        )

            # ---- Phase 3b: logits + softmax + AV per head ----
            bv1 = bias_full1[:].rearrange("(r q h) k -> r q h k", h=H, q=QS // 2)
            bv2 = bias_full2[:].rearrange("(r q h) k -> r q h k", h=H, q=QS // 2)
            with (
                tc.tile_pool(name="lps", bufs=2, space="PSUM") as lps,
                tc.tile_pool(name="trps", bufs=2, space="PSUM") as trps,
                tc.tile_pool(name="avps", bufs=2, space="PSUM") as avps,
                tc.tile_pool(name="battn", bufs=3) as battn,
                tc.tile_pool(name="bexp", bufs=2) as bexp,
                tc.tile_pool(name="bsm", bufs=4) as bsm,
                tc.tile_pool(name="bxp", bufs=2) as bxp,
            ):
                for h in range(H):
                    po = (h % 2) * DH
                    ch = h // 2
                    hs = slice(po, po + DH)
                    expT = bxp.tile([128, 4, L], F32R, tag="expT", name=f"expT_{h}")
                    for qc in range(4):
                        cs = slice(qc * 128, (qc + 1) * 128)
                        ps_l = lps.tile([128, L], F32, tag="lg", name=f"pl_{h}_{qc}")
                        nc.tensor.matmul(
                            ps_l[:],
                            qT_sb[hs, ch, cs],
                            kT_sb[hs, ch, :],
                            start=True,
                            stop=True,
                        )
                        lqk = battn.tile([128, L], F32, tag="lqk", name=f"lq_{h}_{qc}")
                        nc.scalar.activation(lqk[:], ps_l[:], AF.Copy)
                        bias_t = battn.tile(
                            [128, L], F32, tag="biast", name=f"bt_{h}_{qc}"
                        )
                        for rr in range(2):
                            r = 2 * qc + rr
                            nc.sync.dma_start(
                                bias_t[rr * 64 : rr * 64 + 32, :], bv1[r, :, h, :]
                            )
                            nc.sync.dma_start(
                                bias_t[rr * 64 + 32 : rr * 64 + 64, :], bv2[r, :, h, :]
                            )
                        lsb = battn.tile([128, L], F32, tag="lsb", name=f"ls_{h}_{qc}")
                        nc.vector.tensor_tensor(lsb[:], lqk[:], bias_t[:], op=ADD)
                        exp_t = bexp.tile([128, L], F32, tag="exp", name=f"ex_{h}_{qc}")
                        sums = bsm.tile([128, 1], F32, tag="sums", name=f"sm_{h}_{qc}")
                        nc.scalar.activation(
                            exp_t[:], lsb[:], AF.Exp, accum_out=sums[:]
                        )
                        rc = bsm.tile([128, 1], F32, tag="rc", name=f"rc_{h}_{qc}")
                        nc.vector.reciprocal(rc[:], sums[:])
                        exp_s = bexp.tile(
                            [128, L], F32, tag="exps", name=f"exs_{h}_{qc}"
                        )
                        nc.vector.tensor_scalar_mul(exp_s[:], exp_t[:], rc[:])
                        for kc in range(4):
                            tr = trps.tile(
                                [128, 128], F32, tag="tr", name=f"tr_{h}_{qc}_{kc}"
                            )
                            nc.tensor.transpose(
                                tr[:], exp_s[:, kc * 128 : (kc + 1) * 128], ident[:]
                            )
                            nc.scalar.activation(
                                expT[:, kc, qc * 128 : (qc + 1) * 128], tr[:], AF.Copy
                            )
                    ps_av = avps.tile([DH, L], F32, tag="av", name=f"av_{h}")
                    for kc in range(4):
                        nc.tensor.matmul(
                            ps_av[:],
                            v_sb[:, kc, h * DH : (h + 1) * DH],
                            expT[:, kc, :],
                            start=(kc == 0),
                            stop=(kc == 3),
                        )
                    nc.vector.tensor_copy(attnT[:, h, :], ps_av[:])

                # ---- Phase 3c: output projection (f32r) ----
                with tc.tile_pool(name="ops", bufs=2, space="PSUM") as ops:
                    for tc4 in range(4):
                        out_sb = battn.tile([128, D], F32, tag="osb", name=f"osb_{tc4}")
                        for hf in range(2):
                            ps_o = ops.tile(
                                [128, 384], F32, tag="pso", name=f"pso_{tc4}_{hf}"
                            )
                            sl = slice(hf * 384, (hf + 1) * 384)
                            for h2 in range(H):
                                nc.tensor.matmul(
                                    ps_o[:],
                                    attnT[:, h2, tc4 * 128 : (tc4 + 1) * 128],
                                    Wo_sb[:, h2, sl],
                                    start=(h2 == 0),
                                    stop=(h2 == H - 1),
                                )
                            nc.vector.tensor_tensor(
                                out_sb[:, sl], ps_o[:], bo_sb[:, sl], op=ADD
                            )
                        nc.sync.dma_start(
                            out_d[tc4 * 128 : (tc4 + 1) * 128, :], out_sb[:]
                        )

    nc.compile()
    return nc


def _get_nc():
    if "nc" not in _CACHE:
        _CACHE["nc"] = _build()
    return _CACHE["nc"]


def _hi_lo(a, dt):
    hi = a.astype(dt)
    lo = (a - hi.astype(np.float32)).astype(dt)
    return hi, lo


def kernel(
    query,
    key_value,
    query_coords,
    key_coords,
    Wq,
    bq,
    Wk,
    bk,
    Wv,
    bv,
    Wo,
    bo,
    W1,
    b1,
    W2,
    b2,
):
    import ml_dtypes

    query = np.asarray(query, np.float32)
    key_value = np.asarray(key_value, np.float32)
    query_coords = np.asarray(query_coords, np.float32)
    key_coords = np.asarray(key_coords, np.float32)

    def chunked(w, dt=np.float32):  # [768, X] -> [128, 6, X]
        w = np.asarray(w, dt)
        return np.ascontiguousarray(w.reshape(NCH, 128, -1).transpose(1, 0, 2))

    def pchunk(b):  # [768] -> [128, 6]
        return np.ascontiguousarray(np.asarray(b, np.float32).reshape(NCH, 128).T)

    WqS = chunked(np.asarray(Wq, np.float32) * np.float32(SCALE))
    Wk_l = chunked(Wk)
    Wv_l = chunked(Wv)
    Wo_l = np.ascontiguousarray(
        np.asarray(Wo, np.float32).reshape(H, DH, D).transpose(1, 0, 2)
    )
    W2hi, W2lo = _hi_lo(np.asarray(W2, np.float32), np.float16)
    W2P_l = np.concatenate(
        [chunked(W2hi, np.float16), chunked(W2lo, np.float16)], axis=2
    )
    W1f = np.asarray(W1, np.float32)
    W1hi, W1lo = _hi_lo(W1f, ml_dtypes.bfloat16)
    W1P = np.zeros((128, D), ml_dtypes.bfloat16)
    W1P[0:6] = W1hi
    W1P[6:12] = W1hi
    W1P[12:18] = W1lo
    W1P[18:24] = W1lo
    bqS = pchunk(np.asarray(bq, np.float32) * np.float32(SCALE))
    bk_l = pchunk(bk)
    b1_l = pchunk(b1)
    b2_l = np.ascontiguousarray(np.asarray(b2, np.float32).reshape(H, 1))
    bv_b = np.ascontiguousarray(np.broadcast_to(np.asarray(bv, np.float32), (128, D)))
    bo_b = np.ascontiguousarray(np.broadcast_to(np.asarray(bo, np.float32), (128, D)))

    in_maps = []
    for c in range(NCORES):
        qs = slice(c * QS, (c + 1) * QS)
        delta = query_coords[qs, None, :] - key_coords[None, :, :]
        rel = np.concatenate([delta, np.abs(delta), np.square(delta)], axis=-1)
        relT = rel.reshape(QS * L, 6).T
        rhi, rlo = _hi_lo(relT, ml_dtypes.bfloat16)
        relP = np.zeros((128, QS * L), ml_dtypes.bfloat16)
        relP[0:6] = rhi
        relP[6:12] = rlo
        relP[12:18] = rhi
        relP[18:24] = rlo
        in_maps.append(
            {
                "xqT": np.ascontiguousarray(query[c].T),
                "kvT": np.ascontiguousarray(key_value[c].T),
                "relP": relP,
                "WqS": WqS,
                "Wk": Wk_l,
                "Wv": Wv_l,
                "Wo": Wo_l,
                "W1P": W1P,
                "W2P": W2P_l,
                "bqS": bqS,
                "bk": bk_l,
                "b1": b1_l,
                "b2": b2_l,
                "bvb": bv_b,
                "bob": bo_b,
            }
        )

    nc = _get_nc()
    res = bass_utils.run_bass_kernel_spmd(nc, in_maps, core_ids=list(range(NCORES)))
    out = np.stack([res.results[c]["out"] for c in range(NCORES)], axis=0)
    return out.astype(np.float32)



# revision 23
# speedup vs baseline: 1.0967x; 1.0967x over previous
"""Cross-attention with relative-position-bias MLP on 8 Trainium2 NeuronCores.

Sharding: batch-parallel attention (core c owns batch element c) +
k-sharded bias MLP: core c computes bias rows for keys
{g*128 + c*16 + j : g<4, j<16} so that the AllGather of chunk g yields a
DRAM layout [c, j, h, q] whose flattened (c, j) order IS the global key
order of k-chunk g.  Phase 3 then runs attention fully transposed
(logits as [k, q]): bias tiles load with perfect 2KB descriptors and are
folded into the logits PSUM via an identity-matmul accumulate; exp reads
PSUM directly; AV consumes exp with V in natural [k, dh] layout plus a
ones column that yields the softmax sums for free; per-head normalization
is a reciprocal + K=1 broadcast matmul + one [64,512] multiply.

Precision: bias MLP mm1 bf16 hi/lo packed into K=24 (exact to ~2^-17);
mm2 fp16 (hi only); projections / QK / AV / O in f32r; softmax fp32 exp.

Self-contained: hardcodes all shapes; builds/compiles the Bass kernel on
first call and runs it via bass_utils.run_bass_kernel_spmd on cores 0-7.
"""

import numpy as np

import concourse.bass as bass
import concourse.mybir as mybir
import concourse.tile as tile
from concourse import bacc, bass_utils

F32 = mybir.dt.float32
F32R = mybir.dt.float32r
BF16 = mybir.dt.bfloat16
FP16 = mybir.dt.float16
AF = mybir.ActivationFunctionType
ADD = mybir.AluOpType.add
MULT = mybir.AluOpType.mult

NCORES = 8
B = 8
L = 512
D = 768
H = 12
DH = 64
NCH = D // 128
KS = L // NCORES          # 64 owned keys per core
NSTEP = KS // 2           # 32 phase-1 steps (2 owned keys per step)
NCHUNK = 4                # all-gather chunks (16 owned keys each)
SCALE = DH ** -0.5

_CACHE = {}


def _build(dbg=False):
    nc = bacc.Bacc("TRN2", target_bir_lowering=False, debug=False, num_devices=NCORES)

    xqT_d = nc.dram_tensor("xqT", [D, L], F32R, kind="ExternalInput")
    kvT_d = nc.dram_tensor("kvT", [D, L], F32R, kind="ExternalInput")
    relP_d = nc.dram_tensor("relP", [24, NSTEP * 2 * L], BF16, kind="ExternalInput")
    WqS_d = nc.dram_tensor("WqS", [128, NCH, D], F32R, kind="ExternalInput")
    Wk_d = nc.dram_tensor("Wk", [128, NCH, D], F32R, kind="ExternalInput")
    Wv_d = nc.dram_tensor("Wv", [128, NCH, D], F32R, kind="ExternalInput")
    Wo_d = nc.dram_tensor("Wo", [DH, H, D], F32R, kind="ExternalInput")
    W1P_d = nc.dram_tensor("W1P", [24, D], BF16, kind="ExternalInput")
    W2P_d = nc.dram_tensor("W2P", [128, NCH, H], FP16, kind="ExternalInput")
    bqS_d = nc.dram_tensor("bqS", [128, NCH], F32, kind="ExternalInput")
    bk_d = nc.dram_tensor("bk", [128, NCH], F32, kind="ExternalInput")
    b1_d = nc.dram_tensor("b1", [128, NCH], F32, kind="ExternalInput")
    b2_d = nc.dram_tensor("b2", [H, 1], F32, kind="ExternalInput")
    bv_d = nc.dram_tensor("bvb", [128, D], F32, kind="ExternalInput")
    bo_d = nc.dram_tensor("bob", [128, D], F32, kind="ExternalInput")
    idn_d = nc.dram_tensor("idn", [128, 128], F32R, kind="ExternalInput")
    one_d = nc.dram_tensor("one", [128, 64], F32R, kind="ExternalInput")
    out_d = nc.dram_tensor("out", [L, D], F32, kind="ExternalOutput")
    if dbg:
        dbg_full = nc.dram_tensor(
            "dbg_full", [NCORES, 16, H, L], F32, kind="ExternalOutput"
        )
        dbg_qt = nc.dram_tensor("dbg_qt", [128, NCH, L], F32, kind="ExternalOutput")
        dbg_kt = nc.dram_tensor("dbg_kt", [128, NCH, L], F32, kind="ExternalOutput")
        dbg_v = nc.dram_tensor(
            "dbg_v", [128, 4, H, DH + 1], F32, kind="ExternalOutput"
        )
        dbg_exp = nc.dram_tensor("dbg_exp", [128, L], F32, kind="ExternalOutput")
        dbg_att = nc.dram_tensor("dbg_att", [DH, H, L], F32, kind="ExternalOutput")
        dbg_idn = nc.dram_tensor("dbg_idn", [128, 128], F32, kind="ExternalOutput")
        dbg_ib = nc.dram_tensor("dbg_ib", [128, L], F32, kind="ExternalOutput")

    with tile.TileContext(nc) as tc:
        with (
            tc.tile_pool(name="dram", bufs=1, space="DRAM") as dpool,
            tc.tile_pool(name="persist", bufs=1) as pp,
        ):
            shards = [
                dpool.tile([16, H, L], F32R, name=f"shard{g}") for g in range(NCHUNK)
            ]
            fulls = [
                dpool.tile([NCORES, 16, H, L], F32R, name=f"full{g}",
                           addr_space="Shared")
                for g in range(NCHUNK)
            ]

            # ---- persistent SBUF: weights, biases, identity ----
            W1p_sb = pp.tile([24, D], BF16, name="W1p_sb")
            nc.sync.dma_start(W1p_sb[:], W1P_d[:, :])
            W2P_sb = pp.tile([128, NCH, H], FP16, name="W2P_sb")
            nc.sync.dma_start(W2P_sb[:], W2P_d[:, :, :])
            b1_sb = pp.tile([128, NCH], F32, name="b1_sb")
            nc.sync.dma_start(b1_sb[:], b1_d[:, :])
            b2_sb = pp.tile([H, 1], F32, name="b2_sb")
            nc.sync.dma_start(b2_sb[:], b2_d[:, :])
            bq_sb = pp.tile([128, NCH], F32, name="bq_sb")
            nc.sync.dma_start(bq_sb[:], bqS_d[:, :])
            bk_sb = pp.tile([128, NCH], F32, name="bk_sb")
            nc.sync.dma_start(bk_sb[:], bk_d[:, :])
            bv_sb = pp.tile([128, D], F32, name="bv_sb")
            nc.sync.dma_start(bv_sb[:], bv_d[:, :])
            bo_sb = pp.tile([128, D], F32, name="bo_sb")
            nc.sync.dma_start(bo_sb[:], bo_d[:, :])
            ident = pp.tile([128, 128], F32R, name="ident")
            nc.sync.dma_start(ident[:], idn_d[:, :])
            ones1 = pp.tile([1, DH], F32R, name="ones1")
            nc.sync.dma_start(ones1[:], one_d[0:1, :])

            qT_sb = pp.tile([128, NCH, L], F32R, name="qT_sb")
            kT_sb = pp.tile([128, NCH, L], F32R, name="kT_sb")
            # V in natural [k, dh] layout, 65 slots per head (slot 64 = ones)
            v_sb = pp.tile([128, 4, H, DH + 1], F32R, name="v_sb")
            nc.sync.dma_start(
                v_sb[:, :, :, DH:DH + 1].rearrange("p g h o -> p g (h o)"),
                one_d.ap()[:, 0:4 * H].rearrange("p (g h) -> p g h", g=4),
            )
            attnT = pp.tile([DH, H, L], F32R, name="attnT")

            # ---- phase 1 + interleaved projections ----
            with (
                tc.tile_pool(name="p1w", bufs=1) as p1w,
                tc.tile_pool(name="p1rel", bufs=3) as p1rel,
                tc.tile_pool(name="p1gel", bufs=3) as p1gel,
                tc.tile_pool(name="p1out", bufs=3) as p1out,
                tc.tile_pool(name="p1ps", bufs=2, space="PSUM") as p1ps,
                tc.tile_pool(name="p1psb", bufs=2, space="PSUM") as p1psb,
            ):
                WqS_sb = p1w.tile([128, NCH, D], F32R, name="WqS_sb")
                nc.sync.dma_start(WqS_sb[:], WqS_d[:, :, :])
                Wk_sb = p1w.tile([128, NCH, D], F32R, name="Wk_sb")
                nc.sync.dma_start(Wk_sb[:], Wk_d[:, :, :])
                Wv_sb = p1w.tile([128, NCH, D], F32R, name="Wv_sb")
                nc.sync.dma_start(Wv_sb[:], Wv_d[:, :, :])
                xqT_sb = p1w.tile([128, NCH, L], F32R, name="xqT_sb")
                nc.sync.dma_start(
                    xqT_sb[:], xqT_d.ap().rearrange("(c p) t -> p c t", p=128)
                )
                kvT_sb = p1w.tile([128, NCH, L], F32R, name="kvT_sb")
                nc.sync.dma_start(
                    kvT_sb[:], kvT_d.ap().rearrange("(c p) t -> p c t", p=128)
                )

                def proj_units():
                    # q / k projections: out [128 (oc dims), 512]
                    for W_sb, x_sb, b_sb, out_t, pfx in (
                        (WqS_sb, xqT_sb, bq_sb, qT_sb, "q"),
                        (Wk_sb, kvT_sb, bk_sb, kT_sb, "k"),
                    ):
                        for oc in range(NCH):
                            def unit(W_sb=W_sb, x_sb=x_sb, b_sb=b_sb,
                                     out_t=out_t, oc=oc, pfx=pfx):
                                ps = p1ps.tile(
                                    [128, 2 * L], F32, tag="hid",
                                    name=f"pp{pfx}_{oc}",
                                )
                                for di in range(NCH):
                                    nc.tensor.matmul(
                                        ps[:, 0:L],
                                        W_sb[:, di, oc * 128:(oc + 1) * 128],
                                        x_sb[:, di, :],
                                        start=(di == 0),
                                        stop=(di == NCH - 1),
                                    )
                                nc.vector.tensor_scalar_add(
                                    out_t[:, oc, :], ps[:, 0:L],
                                    b_sb[:, oc:oc + 1],
                                )
                            yield unit
                    # v projection: out [128 k-chunk, 6 heads x 64]
                    for tc4 in range(4):
                        for hf in range(2):
                            def unit(tc4=tc4, hf=hf):
                                ps = p1ps.tile(
                                    [128, 2 * L], F32, tag="hid",
                                    name=f"ppv_{tc4}_{hf}",
                                )
                                for di in range(NCH):
                                    nc.tensor.matmul(
                                        ps[:, 0:384],
                                        kvT_sb[:, di, tc4 * 128:(tc4 + 1) * 128],
                                        Wv_sb[:, di, hf * 384:(hf + 1) * 384],
                                        start=(di == 0),
                                        stop=(di == NCH - 1),
                                    )
                                nc.vector.tensor_tensor(
                                    v_sb[:, tc4, 6 * hf:6 * (hf + 1), 0:DH],
                                    ps[:, 0:384].rearrange(
                                        "p (h d) -> p h d", h=6
                                    ),
                                    bv_sb[
                                        :, hf * 384:(hf + 1) * 384
                                    ].rearrange("p (h d) -> p h d", h=6),
                                    op=ADD,
                                )
                            yield unit
                units = proj_units()
                units_done = False

                for s in range(NSTEP):
                    g = s // (NSTEP // NCHUNK)
                    rel2 = p1rel.tile([24, 2 * L], BF16, tag="rel",
                                      name=f"rel_{s}")
                    nc.sync.dma_start(
                        rel2[:], relP_d[:, s * 2 * L:(s + 1) * 2 * L]
                    )
                    bias_ps = p1psb.tile([H, 2 * L], F32, tag="bps",
                                         name=f"bps_{s}")
                    for dc in range(NCH):
                        hid = p1ps.tile([128, 2 * L], F32, tag="hid",
                                        name=f"hid_{s}_{dc}")
                        for j in range(2):
                            nc.tensor.matmul(
                                hid[:, j * L:(j + 1) * L],
                                W1p_sb[:, dc * 128:(dc + 1) * 128],
                                rel2[:, j * L:(j + 1) * L],
                                start=True,
                                stop=True,
                            )
                        gelw = p1gel.tile([128, 2 * L], FP16, tag="gel",
                                          name=f"gel_{s}_{dc}")
                        nc.scalar.activation(
                            gelw[:], hid[:], AF.Gelu, bias=b1_sb[:, dc:dc + 1]
                        )
                        for j in range(2):
                            nc.tensor.matmul(
                                bias_ps[:, j * L:(j + 1) * L],
                                W2P_sb[:, dc, :],
                                gelw[:, j * L:(j + 1) * L],
                                start=(dc == 0),
                                stop=(dc == NCH - 1),
                            )
                    bsb = p1out.tile([H, 2 * L], F32R, tag="bsb", name=f"bsb_{s}")
                    nc.vector.tensor_scalar_add(bsb[:], bias_ps[:], b2_sb[:, 0:1])
                    j0 = (2 * s) % 16
                    nc.sync.dma_start(
                        shards[g][j0:j0 + 2, :, :].rearrange("k h q -> h k q"),
                        bsb[:].rearrange("h (k q) -> h k q", k=2),
                    )
                    # interleave projection work into PE slack
                    if s >= 8 and not units_done:
                        try:
                            next(units)()
                        except StopIteration:
                            units_done = True
                    if s % (NSTEP // NCHUNK) == (NSTEP // NCHUNK) - 1:
                        nc.gpsimd.collective_compute(
                            "AllGather",
                            mybir.AluOpType.bypass,
                            replica_groups=[list(range(NCORES))],
                            ins=[shards[g][:].opt()],
                            outs=[fulls[g][:].opt()],
                        )
                for unit in units:
                    unit()

            # ---- phase 3: transposed attention ----
            with (
                tc.tile_pool(name="p3w", bufs=1) as p3w,
                tc.tile_pool(name="p3b", bufs=12) as p3b,
                tc.tile_pool(name="p3e", bufs=4) as p3e,
                tc.tile_pool(name="p3r", bufs=2) as p3r,
                tc.tile_pool(name="p3o", bufs=2) as p3o,
                tc.tile_pool(name="lps", bufs=4, space="PSUM") as lps,
                tc.tile_pool(name="avps", bufs=2, space="PSUM") as avps,
                tc.tile_pool(name="rbps", bufs=2, space="PSUM") as rbps,
            ):
                Wo_sb = p3w.tile([DH, H, D], F32R, name="Wo_sb")
                nc.sync.dma_start(Wo_sb[:], Wo_d[:, :, :])

                bts = {}

                def load_bias(h):
                    for g in range(NCHUNK):
                        bt = p3b.tile([128, L], F32R, tag="bt",
                                      name=f"bt_{h}_{g}")
                        nc.sync.dma_start(
                            bt[:],
                            fulls[g][:, :, h, :].rearrange("c j q -> (c j) q"),
                        )
                        bts[(h, g)] = bt

                load_bias(0)
                load_bias(1)
                if dbg:
                    nc.sync.dma_start(dbg_idn[:, :], ident[:].bitcast(F32))
                    bt_dbg = p3w.tile([128, L], F32R, name="bt_dbg")
                    nc.sync.dma_start(
                        bt_dbg[:],
                        fulls[0][:, :, 0, :].rearrange("c j q -> (c j) q"),
                    )
                    ps_dbg = lps.tile([128, L], F32, tag="lg", name="ps_dbg")
                    nc.tensor.matmul(
                        ps_dbg[:], ident[:], bt_dbg[:], start=True, stop=True
                    )
                    ib_sb = p3w.tile([128, L], F32, name="ib_sb")
                    nc.scalar.activation(ib_sb[:], ps_dbg[:], AF.Copy)
                    nc.sync.dma_start(dbg_ib[:, :], ib_sb[:])
                for h in range(H):
                    if h + 2 < H:
                        load_bias(h + 2)
                    po = (h % 2) * DH
                    ch = h // 2
                    hs = slice(po, po + DH)
                    av = avps.tile([DH + 1, L], F32, tag="av", name=f"av_{h}")
                    exps = []
                    for g in range(NCHUNK):
                        ps_l = lps.tile([128, L], F32, tag="lg",
                                        name=f"pl_{h}_{g}")
                        nc.tensor.matmul(
                            ps_l[:],
                            kT_sb[hs, ch, g * 128:(g + 1) * 128],
                            qT_sb[hs, ch, :],
                            start=True,
                            stop=False,
                        )
                        nc.tensor.matmul(
                            ps_l[:],
                            ident[:],
                            bts.pop((h, g))[:],
                            start=False,
                            stop=True,
                        )
                        exp_t = p3e.tile([128, L], F32R, tag="exp",
                                         name=f"ex_{h}_{g}")
                        nc.scalar.activation(exp_t[:], ps_l[:], AF.Exp)
                        if dbg and h == 0 and g == 0:
                            nc.sync.dma_start(
                                dbg_exp[:, :], exp_t[:].bitcast(F32)
                            )
                        exps.append(exp_t)
                    for g in range(NCHUNK):
                        nc.tensor.matmul(
                            av[:],
                            v_sb[:, g, h, :],
                            exps[g][:],
                            start=(g == 0),
                            stop=(g == NCHUNK - 1),
                        )
                    rc = p3r.tile([1, L], F32R, tag="rc", name=f"rc_{h}")
                    with nc.allow_low_precision("f32r reciprocal for PE bcast"):
                        nc.vector.reciprocal(rc[:], av[DH:DH + 1, :])
                    rb = rbps.tile([DH, L], F32, tag="rb", name=f"rb_{h}")
                    nc.tensor.matmul(
                        rb[:],
                        ones1[:],
                        rc[:],
                        start=True,
                        stop=True,
                    )
                    rb_sb = p3r.tile([DH, L], F32, tag="rbs", name=f"rbs_{h}")
                    nc.scalar.activation(rb_sb[:], rb[:], AF.Copy)
                    nc.vector.tensor_tensor(
                        attnT[:, h, :], av[0:DH, :], rb_sb[:], op=MULT
                    )

                if dbg:
                    nc.sync.dma_start(dbg_full[:], fulls[0][:].bitcast(F32))
                    nc.sync.dma_start(dbg_qt[:], qT_sb[:].bitcast(F32))
                    nc.sync.dma_start(dbg_kt[:], kT_sb[:].bitcast(F32))
                    nc.sync.dma_start(dbg_v[:], v_sb[:].bitcast(F32))
                    nc.sync.dma_start(dbg_att[:], attnT[:].bitcast(F32))

                # ---- output projection ----
                for tc4 in range(4):
                    out_sb = p3o.tile([128, D], F32, tag="osb",
                                      name=f"osb_{tc4}")
                    for hf in range(2):
                        ps_o = lps.tile([128, L], F32, tag="lg",
                                        name=f"pso_{tc4}_{hf}")
                        sl = slice(hf * 384, (hf + 1) * 384)
                        for h2 in range(H):
                            nc.tensor.matmul(
                                ps_o[:, 0:384],
                                attnT[:, h2, tc4 * 128:(tc4 + 1) * 128],
                                Wo_sb[:, h2, sl],
                                start=(h2 == 0),
                                stop=(h2 == H - 1),
                            )
                        nc.vector.tensor_tensor(
                            out_sb[:, sl], ps_o[:, 0:384], bo_sb[:, sl], op=ADD
                        )
                    nc.sync.dma_start(
                        out_d[tc4 * 128:(tc4 + 1) * 128, :], out_sb[:]
                    )

    nc.compile()
    return nc


def _get_nc():
    if "nc" not in _CACHE:
        _CACHE["nc"] = _build()
    return _CACHE["nc"]


def _hi_lo(a, dt):
    hi = a.astype(dt)
    lo = (a - hi.astype(np.float32)).astype(dt)
    return hi, lo


def _owned_keys(c):
    # core c owns keys {g*128 + c*16 + j}, ordered by (g, j)
    ks = []
    for g in range(NCHUNK):
        for j in range(16):
            ks.append(g * 128 + c * 16 + j)
    return np.array(ks, dtype=np.int64)


def kernel(
    query,
    key_value,
    query_coords,
    key_coords,
    Wq,
    bq,
    Wk,
    bk,
    Wv,
    bv,
    Wo,
    bo,
    W1,
    b1,
    W2,
    b2,
):
    import ml_dtypes

    query = np.asarray(query, np.float32)
    key_value = np.asarray(key_value, np.float32)
    query_coords = np.asarray(query_coords, np.float32)
    key_coords = np.asarray(key_coords, np.float32)

    def chunked(w, dt=np.float32):  # [768, X] -> [128, 6, X]
        w = np.asarray(w, dt)
        return np.ascontiguousarray(w.reshape(NCH, 128, -1).transpose(1, 0, 2))

    def pchunk(b):  # [768] -> [128, 6]
        return np.ascontiguousarray(np.asarray(b, np.float32).reshape(NCH, 128).T)

    WqS = chunked(np.asarray(Wq, np.float32) * np.float32(SCALE))
    Wk_l = chunked(Wk)
    Wv_l = chunked(Wv)
    Wo_l = np.ascontiguousarray(
        np.asarray(Wo, np.float32).reshape(H, DH, D).transpose(1, 0, 2)
    )
    W2P_l = chunked(np.asarray(W2, np.float32), np.float16)  # [128, 6, 12] hi
    W1f = np.asarray(W1, np.float32)
    W1hi, W1lo = _hi_lo(W1f, ml_dtypes.bfloat16)
    W1P = np.zeros((24, D), ml_dtypes.bfloat16)
    W1P[0:6] = W1hi
    W1P[6:12] = W1hi
    W1P[12:18] = W1lo
    W1P[18:24] = W1lo
    bqS = pchunk(np.asarray(bq, np.float32) * np.float32(SCALE))
    bk_l = pchunk(bk)
    b1_l = pchunk(b1)
    b2_l = np.ascontiguousarray(np.asarray(b2, np.float32).reshape(H, 1))
    bv_b = np.ascontiguousarray(np.broadcast_to(np.asarray(bv, np.float32), (128, D)))
    bo_b = np.ascontiguousarray(np.broadcast_to(np.asarray(bo, np.float32), (128, D)))

    in_maps = []
    for c in range(NCORES):
        ks = _owned_keys(c)
        # rel features for (owned k, all q): [64, 512, 6]
        delta = key_coords[ks][:, None, :] * -1.0 + query_coords[None, :, :]
        rel = np.concatenate([delta, np.abs(delta), np.square(delta)], axis=-1)
        relT = rel.reshape(KS * L, 6).T  # [6, 64*512] (k outer, q inner)
        rhi, rlo = _hi_lo(relT, ml_dtypes.bfloat16)
        relP = np.zeros((24, KS * L), ml_dtypes.bfloat16)
        relP[0:6] = rhi
        relP[6:12] = rlo
        relP[12:18] = rhi
        relP[18:24] = rlo
        in_maps.append(
            {
                "xqT": np.ascontiguousarray(query[c].T),
                "kvT": np.ascontiguousarray(key_value[c].T),
                "relP": relP,
                "WqS": WqS,
                "Wk": Wk_l,
                "Wv": Wv_l,
                "Wo": Wo_l,
                "W1P": W1P,
                "W2P": W2P_l,
                "bqS": bqS,
                "bk": bk_l,
                "b1": b1_l,
                "b2": b2_l,
                "bvb": bv_b,
                "bob": bo_b,
                "idn": np.eye(128, dtype=np.float32),
                "one": np.ones((128, 64), dtype=np.float32),
            }
        )

    nc = _get_nc()
    res = bass_utils.run_bass_kernel_spmd(nc, in_maps, core_ids=list(range(NCORES)))
    out = np.stack([res.results[c]["out"] for c in range(NCORES)], axis=0)
    return out.astype(np.float32)


# revision 32
# speedup vs baseline: 1.3078x; 1.1925x over previous
"""Cross-attention with relative-position-bias MLP on 8 Trainium2 NeuronCores.

Sharding: batch-parallel attention (core c owns batch element c) +
k-sharded bias MLP: core c computes bias rows for keys
{g*128 + c*16 + j : g<4, j<16} so that the AllGather of chunk g yields a
DRAM layout [c, j, h, q] whose flattened (c, j) order IS the global key
order of k-chunk g.  Phase 3 runs attention fully transposed (logits as
[k, q]): bias tiles stream in with 2KB descriptors and are folded into
the logits PSUM via an fp16 identity-matmul accumulate; exp reads PSUM
directly (bf16 output - range safe without max subtraction); AV consumes
exp with V in natural [k, dh] layout plus a ones column that yields the
softmax sums for free; per-head normalization is a bf16 reciprocal +
K=1 bf16 broadcast matmul + one [64,512] multiply.

Precision: 16-bit everywhere on the PE (1 cyc/row; f32r lowers to
2-pass fp32 HIGH mode on this stack), fp32 PSUM accumulation. The bias
MLP mm1 uses bf16 hi/lo packed into K=24 (exact to ~2^-17).

Phase-1 software pipeline: mm2 for chunk dc is emitted two dc slots
late so the PE never waits on the gelu chain; projections for phase 3
are interleaved one unit per step as further PE filler (keeps the PE
p-state high: frequent short stalls halve the PE clock).

Self-contained: hardcodes all shapes; builds/compiles the Bass kernel on
first call and runs it via bass_utils.run_bass_kernel_spmd on cores 0-7.
"""

import numpy as np

import concourse.bass as bass
import concourse.mybir as mybir
import concourse.tile as tile
from concourse import bacc, bass_utils

F32 = mybir.dt.float32
F32R = mybir.dt.float32r
BF16 = mybir.dt.bfloat16
FP16 = mybir.dt.float16
AF = mybir.ActivationFunctionType
ADD = mybir.AluOpType.add
MULT = mybir.AluOpType.mult

NCORES = 8
B = 8
L = 512
D = 768
H = 12
DH = 64
NCH = D // 128
KS = L // NCORES          # 64 owned keys per core
NSTEP = KS // 2           # 32 phase-1 steps (2 owned keys per step)
NCHUNK = 4                # all-gather chunks (16 owned keys each)
SCALE = DH ** -0.5

_CACHE = {}


def _build(dbg=False):
    nc = bacc.Bacc("TRN2", target_bir_lowering=False, debug=False, num_devices=NCORES)

    xqT_d = nc.dram_tensor("xqT", [D, L], FP16, kind="ExternalInput")
    kvT_d = nc.dram_tensor("kvT", [D, L], FP16, kind="ExternalInput")
    kvTB_d = nc.dram_tensor("kvTB", [D, L], BF16, kind="ExternalInput")
    relP_d = nc.dram_tensor("relP", [24, NSTEP * 2 * L], BF16, kind="ExternalInput")
    WqS_d = nc.dram_tensor("WqS", [128, NCH, D], FP16, kind="ExternalInput")
    Wk_d = nc.dram_tensor("Wk", [128, NCH, D], FP16, kind="ExternalInput")
    Wv_d = nc.dram_tensor("Wv", [128, NCH, D], BF16, kind="ExternalInput")
    Wo_d = nc.dram_tensor("Wo", [DH, H, D], FP16, kind="ExternalInput")
    W1P_d = nc.dram_tensor("W1P", [24, D], BF16, kind="ExternalInput")
    W2P_d = nc.dram_tensor("W2P", [128, NCH, H], FP16, kind="ExternalInput")
    bqS_d = nc.dram_tensor("bqS", [128, NCH], F32, kind="ExternalInput")
    bk_d = nc.dram_tensor("bk", [128, NCH], F32, kind="ExternalInput")
    b1_d = nc.dram_tensor("b1", [128, NCH], F32, kind="ExternalInput")
    b2_d = nc.dram_tensor("b2", [H, 1], F32, kind="ExternalInput")
    bv_d = nc.dram_tensor("bvb", [128, D], F32, kind="ExternalInput")
    bo_d = nc.dram_tensor("bob", [128, D], F32, kind="ExternalInput")
    idn_d = nc.dram_tensor("idn", [128, 128], FP16, kind="ExternalInput")
    one_d = nc.dram_tensor("one", [1, 64], BF16, kind="ExternalInput")
    oneh_d = nc.dram_tensor("oneh", [128, 4 * H], BF16, kind="ExternalInput")
    out_d = nc.dram_tensor("out", [L, D], F32, kind="ExternalOutput")
    if dbg:
        dbg_full = nc.dram_tensor(
            "dbg_full", [NCORES, 16, H, L], FP16, kind="ExternalOutput"
        )
        dbg_qt = nc.dram_tensor("dbg_qt", [128, NCH, L], FP16, kind="ExternalOutput")
        dbg_kt = nc.dram_tensor("dbg_kt", [128, NCH, L], FP16, kind="ExternalOutput")
        dbg_v = nc.dram_tensor(
            "dbg_v", [128, 4, H, DH + 1], BF16, kind="ExternalOutput"
        )
        dbg_exp = nc.dram_tensor("dbg_exp", [128, L], BF16, kind="ExternalOutput")
        dbg_att = nc.dram_tensor("dbg_att", [DH, H, L], FP16, kind="ExternalOutput")

    with tile.TileContext(nc) as tc:
        with (
            tc.tile_pool(name="dram", bufs=1, space="DRAM") as dpool,
            tc.tile_pool(name="persist", bufs=1) as pp,
        ):
            shards = [
                dpool.tile([16, H, L], FP16, name=f"shard{g}") for g in range(NCHUNK)
            ]
            fulls = [
                dpool.tile([NCORES, 16, H, L], FP16, name=f"full{g}",
                           addr_space="Shared")
                for g in range(NCHUNK)
            ]

            # ---- persistent SBUF: weights, biases, identity ----
            W1p_sb = pp.tile([24, D], BF16, name="W1p_sb")
            nc.sync.dma_start(W1p_sb[:], W1P_d[:, :])
            W2P_sb = pp.tile([128, NCH, H], FP16, name="W2P_sb")
            nc.sync.dma_start(W2P_sb[:], W2P_d[:, :, :])
            b1_sb = pp.tile([128, NCH], F32, name="b1_sb")
            nc.sync.dma_start(b1_sb[:], b1_d[:, :])
            b2_sb = pp.tile([H, 1], F32, name="b2_sb")
            nc.sync.dma_start(b2_sb[:], b2_d[:, :])
            bq_sb = pp.tile([128, NCH], F32, name="bq_sb")
            nc.sync.dma_start(bq_sb[:], bqS_d[:, :])
            bk_sb = pp.tile([128, NCH], F32, name="bk_sb")
            nc.sync.dma_start(bk_sb[:], bk_d[:, :])
            bv_sb = pp.tile([128, D], F32, name="bv_sb")
            nc.sync.dma_start(bv_sb[:], bv_d[:, :])
            bo_sb = pp.tile([128, D], F32, name="bo_sb")
            nc.sync.dma_start(bo_sb[:], bo_d[:, :])
            ident = pp.tile([128, 128], FP16, name="ident")
            nc.sync.dma_start(ident[:], idn_d[:, :])
            ones1 = pp.tile([1, DH], BF16, name="ones1")
            nc.sync.dma_start(ones1[:], one_d[0:1, :])

            qT_sb = pp.tile([128, NCH, L], FP16, name="qT_sb")
            kT_sb = pp.tile([128, NCH, L], FP16, name="kT_sb")
            # V in natural [k, dh] layout, 65 slots per head (slot 64 = ones)
            v_sb = pp.tile([128, 4, H, DH + 1], BF16, name="v_sb")
            nc.sync.dma_start(
                v_sb[:, :, :, DH:DH + 1].rearrange("p g h o -> p g (h o)"),
                oneh_d.ap().rearrange("p (g h) -> p g h", g=4),
            )
            attnT = pp.tile([DH, H, L], FP16, name="attnT")

            # ---- phase 1 + interleaved projections ----
            with (
                tc.tile_pool(name="p1w", bufs=1) as p1w,
                tc.tile_pool(name="p1rel", bufs=3) as p1rel,
                tc.tile_pool(name="p1gel", bufs=4) as p1gel,
                tc.tile_pool(name="p1out", bufs=3) as p1out,
                tc.tile_pool(name="p1ps", bufs=2, space="PSUM") as p1ps,
                tc.tile_pool(name="p1psb", bufs=2, space="PSUM") as p1psb,
            ):
                WqS_sb = p1w.tile([128, NCH, D], FP16, name="WqS_sb")
                nc.sync.dma_start(WqS_sb[:], WqS_d[:, :, :])
                Wk_sb = p1w.tile([128, NCH, D], FP16, name="Wk_sb")
                nc.sync.dma_start(Wk_sb[:], Wk_d[:, :, :])
                Wv_sb = p1w.tile([128, NCH, D], BF16, name="Wv_sb")
                nc.sync.dma_start(Wv_sb[:], Wv_d[:, :, :])
                xqT_sb = p1w.tile([128, NCH, L], FP16, name="xqT_sb")
                nc.sync.dma_start(
                    xqT_sb[:], xqT_d.ap().rearrange("(c p) t -> p c t", p=128)
                )
                kvT_sb = p1w.tile([128, NCH, L], FP16, name="kvT_sb")
                nc.sync.dma_start(
                    kvT_sb[:], kvT_d.ap().rearrange("(c p) t -> p c t", p=128)
                )
                kvTb_sb = p1w.tile([128, NCH, L], BF16, name="kvTb_sb")
                nc.sync.dma_start(
                    kvTb_sb[:], kvTB_d.ap().rearrange("(c p) t -> p c t", p=128)
                )

                def proj_units():
                    # q / k projections: out [128 (oc dims), 512]
                    for W_sb, x_sb, b_sb, out_t, pfx in (
                        (WqS_sb, xqT_sb, bq_sb, qT_sb, "q"),
                        (Wk_sb, kvT_sb, bk_sb, kT_sb, "k"),
                    ):
                        for oc in range(NCH):
                            def unit(W_sb=W_sb, x_sb=x_sb, b_sb=b_sb,
                                     out_t=out_t, oc=oc, pfx=pfx):
                                ps = p1ps.tile(
                                    [128, 2 * L], F32, tag="hid",
                                    name=f"pp{pfx}_{oc}",
                                )
                                for di in range(NCH):
                                    nc.tensor.matmul(
                                        ps[:, 0:L],
                                        W_sb[:, di, oc * 128:(oc + 1) * 128],
                                        x_sb[:, di, :],
                                        start=(di == 0),
                                        stop=(di == NCH - 1),
                                    )
                                nc.vector.tensor_scalar_add(
                                    out_t[:, oc, :], ps[:, 0:L],
                                    b_sb[:, oc:oc + 1],
                                )
                            yield unit
                    # v projection: out [128 k-chunk, 6 heads x 64]
                    for tc4 in range(4):
                        for hf in range(2):
                            def unit(tc4=tc4, hf=hf):
                                ps = p1ps.tile(
                                    [128, 2 * L], F32, tag="hid",
                                    name=f"ppv_{tc4}_{hf}",
                                )
                                for di in range(NCH):
                                    nc.tensor.matmul(
                                        ps[:, 0:384],
                                        kvTb_sb[:, di, tc4 * 128:(tc4 + 1) * 128],
                                        Wv_sb[:, di, hf * 384:(hf + 1) * 384],
                                        start=(di == 0),
                                        stop=(di == NCH - 1),
                                    )
                                nc.vector.tensor_tensor(
                                    v_sb[:, tc4, 6 * hf:6 * (hf + 1), 0:DH],
                                    ps[:, 0:384].rearrange(
                                        "p (h d) -> p h d", h=6
                                    ),
                                    bv_sb[
                                        :, hf * 384:(hf + 1) * 384
                                    ].rearrange("p (h d) -> p h d", h=6),
                                    op=ADD,
                                )
                            yield unit
                units = proj_units()
                units_done = False

                # software pipeline: pending mm2 emissions (2 dc slots late)
                from collections import deque
                pend = deque()

                def emit_pend():
                    fn = pend.popleft()
                    fn()

                bias_tiles = {}

                def finish_step(s):
                    # b2 add + store to the shard + maybe all-gather
                    g = s // (NSTEP // NCHUNK)
                    bias_ps = bias_tiles.pop(s)
                    bsb = p1out.tile([H, 2 * L], FP16, tag="bsb",
                                     name=f"bsb_{s}")
                    nc.vector.tensor_scalar_add(bsb[:], bias_ps[:], b2_sb[:, 0:1])
                    j0 = (2 * s) % 16
                    nc.sync.dma_start(
                        shards[g][j0:j0 + 2, :, :].rearrange("k h q -> h k q"),
                        bsb[:].rearrange("h (k q) -> h k q", k=2),
                    )
                    if s % (NSTEP // NCHUNK) == (NSTEP // NCHUNK) - 1:
                        nc.gpsimd.collective_compute(
                            "AllGather",
                            mybir.AluOpType.bypass,
                            replica_groups=[list(range(NCORES))],
                            ins=[shards[g][:].opt()],
                            outs=[fulls[g][:].opt()],
                        )

                for s in range(NSTEP):
                    rel2 = p1rel.tile([24, 2 * L], BF16, tag="rel",
                                      name=f"rel_{s}")
                    nc.sync.dma_start(
                        rel2[:], relP_d[:, s * 2 * L:(s + 1) * 2 * L]
                    )
                    bias_ps = p1psb.tile([H, 2 * L], F32, tag="bps",
                                         name=f"bps_{s}")
                    bias_tiles[s] = bias_ps
                    for dc in range(NCH):
                        hid = p1ps.tile([128, 2 * L], F32, tag="hid",
                                        name=f"hid_{s}_{dc}")
                        for j in range(2):
                            nc.tensor.matmul(
                                hid[:, j * L:(j + 1) * L],
                                W1p_sb[:, dc * 128:(dc + 1) * 128],
                                rel2[:, j * L:(j + 1) * L],
                                start=True,
                                stop=True,
                            )
                        gelw = p1gel.tile([128, 2 * L], FP16, tag="gel",
                                          name=f"gel_{s}_{dc}")
                        nc.scalar.activation(
                            gelw[:], hid[:], AF.Gelu, bias=b1_sb[:, dc:dc + 1]
                        )

                        def mm2(s=s, dc=dc, bias_ps=bias_ps, gelw=gelw):
                            for j in range(2):
                                nc.tensor.matmul(
                                    bias_ps[:, j * L:(j + 1) * L],
                                    W2P_sb[:, dc, :],
                                    gelw[:, j * L:(j + 1) * L],
                                    start=(dc == 0),
                                    stop=(dc == NCH - 1),
                                )
                            if dc == NCH - 1:
                                finish_step(s)
                        pend.append(mm2)
                        if len(pend) > 2:
                            emit_pend()
                    # interleave projection work into PE slack
                    if s >= 8 and not units_done:
                        try:
                            next(units)()
                        except StopIteration:
                            units_done = True
                while pend:
                    emit_pend()
                for unit in units:
                    unit()

            # ---- phase 3: transposed attention ----
            with (
                tc.tile_pool(name="p3w", bufs=1) as p3w,
                tc.tile_pool(name="p3b", bufs=12) as p3b,
                tc.tile_pool(name="p3e", bufs=4) as p3e,
                tc.tile_pool(name="p3r", bufs=2) as p3r,
                tc.tile_pool(name="p3o", bufs=2) as p3o,
                tc.tile_pool(name="lps", bufs=4, space="PSUM") as lps,
                tc.tile_pool(name="avps", bufs=2, space="PSUM") as avps,
                tc.tile_pool(name="rbps", bufs=2, space="PSUM") as rbps,
            ):
                Wo_sb = p3w.tile([DH, H, D], FP16, name="Wo_sb")
                nc.sync.dma_start(Wo_sb[:], Wo_d[:, :, :])

                bts = {}

                def load_bias(h):
                    for g in range(NCHUNK):
                        bt = p3b.tile([128, L], FP16, tag="bt",
                                      name=f"bt_{h}_{g}")
                        nc.sync.dma_start(
                            bt[:],
                            fulls[g][:, :, h, :].rearrange("c j q -> (c j) q"),
                        )
                        bts[(h, g)] = bt

                def qkb(h, g):
                    po = (h % 2) * DH
                    ch = h // 2
                    hs = slice(po, po + DH)
                    ps_l = lps.tile([128, L], F32, tag="lg", name=f"pl_{h}_{g}")
                    nc.tensor.matmul(
                        ps_l[:],
                        kT_sb[hs, ch, g * 128:(g + 1) * 128],
                        qT_sb[hs, ch, :],
                        start=True,
                        stop=False,
                    )
                    nc.tensor.matmul(
                        ps_l[:],
                        ident[:],
                        bts.pop((h, g))[:],
                        start=False,
                        stop=True,
                    )
                    exp_t = p3e.tile([128, L], BF16, tag="exp",
                                     name=f"ex_{h}_{g}")
                    nc.scalar.activation(exp_t[:], ps_l[:], AF.Exp)
                    if dbg and h == 0 and g == 0:
                        nc.sync.dma_start(dbg_exp[:, :], exp_t[:])
                    return exp_t

                load_bias(0)
                load_bias(1)
                for h in range(H):
                    if h + 2 < H:
                        load_bias(h + 2)
                    av = avps.tile([DH + 1, L], F32, tag="av", name=f"av_{h}")
                    exps = [qkb(h, g) for g in range(NCHUNK - 1)]
                    nc.tensor.matmul(
                        av[:], v_sb[:, 0, h, :], exps[0][:],
                        start=True, stop=False,
                    )
                    # g=3 last: gated on the final all-gather
                    exp3 = qkb(h, NCHUNK - 1)
                    for g in range(1, NCHUNK - 1):
                        nc.tensor.matmul(
                            av[:], v_sb[:, g, h, :], exps[g][:],
                            start=False, stop=False,
                        )
                    nc.tensor.matmul(
                        av[:], v_sb[:, NCHUNK - 1, h, :], exp3[:],
                        start=False, stop=True,
                    )
                    rc = p3r.tile([1, L], BF16, tag="rc", name=f"rc_{h}")
                    with nc.allow_low_precision("bf16 softmax scale bcast"):
                        nc.vector.reciprocal(rc[:], av[DH:DH + 1, :])
                    rb = rbps.tile([DH, L], F32, tag="rb", name=f"rb_{h}")
                    nc.tensor.matmul(
                        rb[:], ones1[:], rc[:], start=True, stop=True,
                    )
                    rb_sb = p3r.tile([DH, L], F32, tag="rbs", name=f"rbs_{h}")
                    nc.vector.tensor_copy(rb_sb[:], rb[:])
                    nc.vector.tensor_tensor(
                        attnT[:, h, :], av[0:DH, :], rb_sb[:], op=MULT
                    )

                if dbg:
                    nc.sync.dma_start(dbg_full[:], fulls[0][:])
                    nc.sync.dma_start(dbg_qt[:], qT_sb[:])
                    nc.sync.dma_start(dbg_kt[:], kT_sb[:])
                    nc.sync.dma_start(dbg_v[:], v_sb[:])
                    nc.sync.dma_start(dbg_att[:], attnT[:])

                # ---- output projection ----
                for tc4 in range(4):
                    out_sb = p3o.tile([128, D], F32, tag="osb",
                                      name=f"osb_{tc4}")
                    for hf in range(2):
                        ps_o = lps.tile([128, L], F32, tag="lg",
                                        name=f"pso_{tc4}_{hf}")
                        sl = slice(hf * 384, (hf + 1) * 384)
                        for h2 in range(H):
                            nc.tensor.matmul(
                                ps_o[:, 0:384],
                                attnT[:, h2, tc4 * 128:(tc4 + 1) * 128],
                                Wo_sb[:, h2, sl],
                                start=(h2 == 0),
                                stop=(h2 == H - 1),
                            )
                        nc.vector.tensor_tensor(
                            out_sb[:, sl], ps_o[:, 0:384], bo_sb[:, sl], op=ADD
                        )
                    nc.sync.dma_start(
                        out_d[tc4 * 128:(tc4 + 1) * 128, :], out_sb[:]
                    )

    nc.compile()
    return nc


def _get_nc():
    if "nc" not in _CACHE:
        _CACHE["nc"] = _build()
    return _CACHE["nc"]


def _hi_lo(a, dt):
    hi = a.astype(dt)
    lo = (a - hi.astype(np.float32)).astype(dt)
    return hi, lo


def _owned_keys(c):
    # core c owns keys {g*128 + c*16 + j}, ordered by (g, j)
    ks = []
    for g in range(NCHUNK):
        for j in range(16):
            ks.append(g * 128 + c * 16 + j)
    return np.array(ks, dtype=np.int64)


def kernel(
    query,
    key_value,
    query_coords,
    key_coords,
    Wq,
    bq,
    Wk,
    bk,
    Wv,
    bv,
    Wo,
    bo,
    W1,
    b1,
    W2,
    b2,
):
    import ml_dtypes

    query = np.asarray(query, np.float32)
    key_value = np.asarray(key_value, np.float32)
    query_coords = np.asarray(query_coords, np.float32)
    key_coords = np.asarray(key_coords, np.float32)

    def chunked(w, dt=np.float16):  # [768, X] -> [128, 6, X]
        w = np.asarray(w, np.float32).astype(dt)
        return np.ascontiguousarray(w.reshape(NCH, 128, -1).transpose(1, 0, 2))

    def pchunk(b):  # [768] -> [128, 6]
        return np.ascontiguousarray(np.asarray(b, np.float32).reshape(NCH, 128).T)

    WqS = chunked(np.asarray(Wq, np.float32) * np.float32(SCALE))
    Wk_l = chunked(Wk)
    Wv_l = chunked(Wv, ml_dtypes.bfloat16)
    Wo_l = np.ascontiguousarray(
        np.asarray(Wo, np.float32).astype(np.float16)
        .reshape(H, DH, D).transpose(1, 0, 2)
    )
    W2P_l = chunked(W2)  # [128, 6, 12] fp16
    W1f = np.asarray(W1, np.float32)
    W1hi, W1lo = _hi_lo(W1f, ml_dtypes.bfloat16)
    W1P = np.zeros((24, D), ml_dtypes.bfloat16)
    W1P[0:6] = W1hi
    W1P[6:12] = W1hi
    W1P[12:18] = W1lo
    W1P[18:24] = W1lo
    bqS = pchunk(np.asarray(bq, np.float32) * np.float32(SCALE))
    bk_l = pchunk(bk)
    b1_l = pchunk(b1)
    b2_l = np.ascontiguousarray(np.asarray(b2, np.float32).reshape(H, 1))
    bv_b = np.ascontiguousarray(np.broadcast_to(np.asarray(bv, np.float32), (128, D)))
    bo_b = np.ascontiguousarray(np.broadcast_to(np.asarray(bo, np.float32), (128, D)))

    in_maps = []
    for c in range(NCORES):
        ks = _owned_keys(c)
        # rel features for (owned k, all q): [64, 512, 6]
        delta = key_coords[ks][:, None, :] * -1.0 + query_coords[None, :, :]
        rel = np.concatenate([delta, np.abs(delta), np.square(delta)], axis=-1)
        relT = rel.reshape(KS * L, 6).T  # [6, 64*512] (k outer, q inner)
        rhi, rlo = _hi_lo(relT, ml_dtypes.bfloat16)
        relP = np.zeros((24, KS * L), ml_dtypes.bfloat16)
        relP[0:6] = rhi
        relP[6:12] = rlo
        relP[12:18] = rhi
        relP[18:24] = rlo
        in_maps.append(
            {
                "xqT": np.ascontiguousarray(query[c].T).astype(np.float16),
                "kvT": np.ascontiguousarray(key_value[c].T).astype(np.float16),
                "kvTB": np.ascontiguousarray(key_value[c].T).astype(
                    ml_dtypes.bfloat16
                ),
                "relP": relP,
                "WqS": WqS,
                "Wk": Wk_l,
                "Wv": Wv_l,
                "Wo": Wo_l,
                "W1P": W1P,
                "W2P": W2P_l,
                "bqS": bqS,
                "bk": bk_l,
                "b1": b1_l,
                "b2": b2_l,
                "bvb": bv_b,
                "bob": bo_b,
                "idn": np.eye(128, dtype=np.float16),
                "one": np.ones((1, 64), dtype=ml_dtypes.bfloat16),
                "oneh": np.ones((128, 4 * H), dtype=ml_dtypes.bfloat16),
            }
        )

    nc = _get_nc()
    res = bass_utils.run_bass_kernel_spmd(nc, in_maps, core_ids=list(range(NCORES)))
    out = np.stack([res.results[c]["out"] for c in range(NCORES)], axis=0)
    return out.astype(np.float32)


# revision 34
# speedup vs baseline: 1.3111x; 1.0025x over previous
"""Cross-attention with relative-position-bias MLP on 8 Trainium2 NeuronCores.

Sharding: batch-parallel attention (core c owns batch element c) +
k-sharded bias MLP: core c computes bias rows for keys
{g*64 + c*8 + j : g<8, j<8} so that the AllGather of chunk g yields a
DRAM layout [c, j, h, q] whose flattened (c, j) order IS the global key
order of the 64-key block g.  Phase 3 runs attention fully transposed
(logits as [k, q]): bias tiles stream in with 2KB descriptors (two
64-partition halves per 128-k tile) and are folded into the logits PSUM
via an fp16 identity-matmul accumulate; exp reads PSUM directly (bf16
output - range safe without max subtraction); AV consumes exp with V in
natural [k, dh] layout plus a ones column that yields the softmax sums
for free; per-head normalization is a bf16 reciprocal + K=1 bf16
broadcast matmul + one [64,512] multiply.

Precision: 16-bit everywhere on the PE (1 cyc/row; f32r lowers to
2-pass fp32 HIGH mode on this stack), fp32 PSUM accumulation. The bias
MLP mm1 uses bf16 hi/lo packed into K=24 (exact to ~2^-17).

Scheduling: phase-1 mm2 runs two dc slots late (software pipeline) and
phase-3 projections interleave one matmul per dc slot so the PE never
idles on the gelu chain (frequent short stalls halve the PE clock);
phase-3 heads run in a 2-stage pipeline so QK work of head h+1 covers
the exp latency of head h. Big weight loads ride the ACT/Pool DMA
queues so the Sync queue serves the latency-critical rel/shard/bias
tiles.

Self-contained: hardcodes all shapes; builds/compiles the Bass kernel on
first call and runs it via bass_utils.run_bass_kernel_spmd on cores 0-7.
"""

import numpy as np

import concourse.bass as bass
import concourse.mybir as mybir
import concourse.tile as tile
from concourse import bacc, bass_utils

F32 = mybir.dt.float32
F32R = mybir.dt.float32r
BF16 = mybir.dt.bfloat16
FP16 = mybir.dt.float16
AF = mybir.ActivationFunctionType
ADD = mybir.AluOpType.add
MULT = mybir.AluOpType.mult

NCORES = 8
B = 8
L = 512
D = 768
H = 12
DH = 64
NCH = D // 128
KS = L // NCORES          # 64 owned keys per core
NSTEP = KS // 2           # 32 phase-1 steps (2 owned keys per step)
NCHUNK = 8                # all-gather chunks (8 owned keys each)
SPC = NSTEP // NCHUNK     # steps per chunk = 4
SCALE = DH ** -0.5

_CACHE = {}


def _build(dbg=False):
    nc = bacc.Bacc("TRN2", target_bir_lowering=False, debug=False, num_devices=NCORES)

    xqT_d = nc.dram_tensor("xqT", [D, L], FP16, kind="ExternalInput")
    kvT_d = nc.dram_tensor("kvT", [D, L], FP16, kind="ExternalInput")
    kvTB_d = nc.dram_tensor("kvTB", [D, L], BF16, kind="ExternalInput")
    relP_d = nc.dram_tensor("relP", [24, NSTEP * 2 * L], BF16, kind="ExternalInput")
    WqS_d = nc.dram_tensor("WqS", [128, NCH, D], FP16, kind="ExternalInput")
    Wk_d = nc.dram_tensor("Wk", [128, NCH, D], FP16, kind="ExternalInput")
    Wv_d = nc.dram_tensor("Wv", [128, NCH, D], BF16, kind="ExternalInput")
    Wo_d = nc.dram_tensor("Wo", [DH, H, D], FP16, kind="ExternalInput")
    W1P_d = nc.dram_tensor("W1P", [24, D], BF16, kind="ExternalInput")
    W2P_d = nc.dram_tensor("W2P", [128, NCH, H], FP16, kind="ExternalInput")
    bqS_d = nc.dram_tensor("bqS", [128, NCH], F32, kind="ExternalInput")
    bk_d = nc.dram_tensor("bk", [128, NCH], F32, kind="ExternalInput")
    b1_d = nc.dram_tensor("b1", [128, NCH], F32, kind="ExternalInput")
    b2_d = nc.dram_tensor("b2", [H, 1], F32, kind="ExternalInput")
    bv_d = nc.dram_tensor("bvb", [128, D], F32, kind="ExternalInput")
    bo_d = nc.dram_tensor("bob", [128, D], F32, kind="ExternalInput")
    idn_d = nc.dram_tensor("idn", [128, 128], FP16, kind="ExternalInput")
    one_d = nc.dram_tensor("one", [1, 64], BF16, kind="ExternalInput")
    oneh_d = nc.dram_tensor("oneh", [128, 4 * H], BF16, kind="ExternalInput")
    out_d = nc.dram_tensor("out", [L, D], F32, kind="ExternalOutput")
    if dbg:
        dbg_full = nc.dram_tensor(
            "dbg_full", [NCORES, 8, H, L], FP16, kind="ExternalOutput"
        )
        dbg_qt = nc.dram_tensor("dbg_qt", [128, NCH, L], FP16, kind="ExternalOutput")
        dbg_exp = nc.dram_tensor("dbg_exp", [128, L], BF16, kind="ExternalOutput")
        dbg_att = nc.dram_tensor("dbg_att", [DH, H, L], FP16, kind="ExternalOutput")

    with tile.TileContext(nc) as tc:
        with (
            tc.tile_pool(name="dram", bufs=1, space="DRAM") as dpool,
            tc.tile_pool(name="persist", bufs=1) as pp,
        ):
            shards = [
                dpool.tile([8, H, L], FP16, name=f"shard{g}") for g in range(NCHUNK)
            ]
            fulls = [
                dpool.tile([NCORES, 8, H, L], FP16, name=f"full{g}",
                           addr_space="Shared")
                for g in range(NCHUNK)
            ]

            # ---- persistent SBUF (phase-1-critical small tiles on sync) ----
            W1p_sb = pp.tile([24, D], BF16, name="W1p_sb")
            nc.sync.dma_start(W1p_sb[:], W1P_d[:, :])
            W2P_sb = pp.tile([128, NCH, H], FP16, name="W2P_sb")
            nc.sync.dma_start(W2P_sb[:], W2P_d[:, :, :])
            b1_sb = pp.tile([128, NCH], F32, name="b1_sb")
            nc.sync.dma_start(b1_sb[:], b1_d[:, :])
            b2_sb = pp.tile([H, 1], F32, name="b2_sb")
            nc.sync.dma_start(b2_sb[:], b2_d[:, :])
            # rest of the constants ride the Pool-engine DMA queue
            bq_sb = pp.tile([128, NCH], F32, name="bq_sb")
            nc.gpsimd.dma_start(bq_sb[:], bqS_d[:, :])
            bk_sb = pp.tile([128, NCH], F32, name="bk_sb")
            nc.gpsimd.dma_start(bk_sb[:], bk_d[:, :])
            bv_sb = pp.tile([128, D], F32, name="bv_sb")
            nc.gpsimd.dma_start(bv_sb[:], bv_d[:, :])
            bo_sb = pp.tile([128, D], F32, name="bo_sb")
            nc.gpsimd.dma_start(bo_sb[:], bo_d[:, :])
            ident = pp.tile([128, 128], FP16, name="ident")
            nc.gpsimd.dma_start(ident[:], idn_d[:, :])
            ones1 = pp.tile([1, DH], BF16, name="ones1")
            nc.gpsimd.dma_start(ones1[:], one_d[0:1, :])

            qT_sb = pp.tile([128, NCH, L], FP16, name="qT_sb")
            kT_sb = pp.tile([128, NCH, L], FP16, name="kT_sb")
            # V in natural [k, dh] layout, 65 slots per head (slot 64 = ones)
            v_sb = pp.tile([128, 4, H, DH + 1], BF16, name="v_sb")
            nc.gpsimd.dma_start(
                v_sb[:, :, :, DH:DH + 1].rearrange("p g h o -> p g (h o)"),
                oneh_d.ap().rearrange("p (g h) -> p g h", g=4),
            )
            attnT = pp.tile([DH, H, L], FP16, name="attnT")
            Wo_sb = pp.tile([DH, H, D], FP16, name="Wo_sb")
            nc.gpsimd.dma_start(Wo_sb[:], Wo_d[:, :, :])

            # ---- phase 1 + interleaved projections ----
            with (
                tc.tile_pool(name="p1w", bufs=1) as p1w,
                tc.tile_pool(name="p1rel", bufs=3) as p1rel,
                tc.tile_pool(name="p1gel", bufs=4) as p1gel,
                tc.tile_pool(name="p1out", bufs=3) as p1out,
                tc.tile_pool(name="p1ps", bufs=2, space="PSUM") as p1ps,
                tc.tile_pool(name="p1psb", bufs=1, space="PSUM") as p1psb,
                tc.tile_pool(name="p1pj", bufs=2, space="PSUM") as p1pj,
            ):
                # activation-engine DMA queue: big projection operands
                WqS_sb = p1w.tile([128, NCH, D], FP16, name="WqS_sb")
                nc.scalar.dma_start(WqS_sb[:], WqS_d[:, :, :])
                xqT_sb = p1w.tile([128, NCH, L], FP16, name="xqT_sb")
                nc.scalar.dma_start(
                    xqT_sb[:], xqT_d.ap().rearrange("(c p) t -> p c t", p=128)
                )
                Wk_sb = p1w.tile([128, NCH, D], FP16, name="Wk_sb")
                nc.scalar.dma_start(Wk_sb[:], Wk_d[:, :, :])
                kvT_sb = p1w.tile([128, NCH, L], FP16, name="kvT_sb")
                nc.scalar.dma_start(
                    kvT_sb[:], kvT_d.ap().rearrange("(c p) t -> p c t", p=128)
                )
                Wv_sb = p1w.tile([128, NCH, D], BF16, name="Wv_sb")
                nc.scalar.dma_start(Wv_sb[:], Wv_d[:, :, :])
                kvTb_sb = p1w.tile([128, NCH, L], BF16, name="kvTb_sb")
                nc.scalar.dma_start(
                    kvTb_sb[:], kvTB_d.ap().rearrange("(c p) t -> p c t", p=128)
                )

                def proj_microops():
                    # q / k projections: out [128 (oc dims), 512]
                    for W_sb, x_sb, b_sb, out_t, pfx in (
                        (WqS_sb, xqT_sb, bq_sb, qT_sb, "q"),
                        (Wk_sb, kvT_sb, bk_sb, kT_sb, "k"),
                    ):
                        for oc in range(NCH):
                            cell = {}

                            def mk_mm(di, cell=cell, W_sb=W_sb, x_sb=x_sb,
                                      oc=oc, pfx=pfx):
                                def op():
                                    if di == 0:
                                        cell["ps"] = p1pj.tile(
                                            [128, L], F32, tag="pj",
                                            name=f"pp{pfx}_{oc}",
                                        )
                                    nc.tensor.matmul(
                                        cell["ps"][:],
                                        W_sb[:, di, oc * 128:(oc + 1) * 128],
                                        x_sb[:, di, :],
                                        start=(di == 0),
                                        stop=(di == NCH - 1),
                                    )
                                return op
                            for di in range(NCH):
                                yield mk_mm(di)

                            def fin(cell=cell, b_sb=b_sb, out_t=out_t, oc=oc):
                                nc.vector.tensor_scalar_add(
                                    out_t[:, oc, :], cell["ps"][:],
                                    b_sb[:, oc:oc + 1],
                                )
                            yield fin
                    # v projection: out [128 k-chunk, 6 heads x 64]
                    for tc4 in range(4):
                        for hf in range(2):
                            cell = {}

                            def mk_mm(di, cell=cell, tc4=tc4, hf=hf):
                                def op():
                                    if di == 0:
                                        cell["ps"] = p1pj.tile(
                                            [128, L], F32, tag="pj",
                                            name=f"ppv_{tc4}_{hf}",
                                        )
                                    nc.tensor.matmul(
                                        cell["ps"][:, 0:384],
                                        kvTb_sb[:, di,
                                                tc4 * 128:(tc4 + 1) * 128],
                                        Wv_sb[:, di, hf * 384:(hf + 1) * 384],
                                        start=(di == 0),
                                        stop=(di == NCH - 1),
                                    )
                                return op
                            for di in range(NCH):
                                yield mk_mm(di)

                            def fin(cell=cell, tc4=tc4, hf=hf):
                                nc.vector.tensor_tensor(
                                    v_sb[:, tc4, 6 * hf:6 * (hf + 1), 0:DH],
                                    cell["ps"][:, 0:384].rearrange(
                                        "p (h d) -> p h d", h=6
                                    ),
                                    bv_sb[
                                        :, hf * 384:(hf + 1) * 384
                                    ].rearrange("p (h d) -> p h d", h=6),
                                    op=ADD,
                                )
                            yield fin
                micro = proj_microops()
                micro_done = False

                from collections import deque
                pend = deque()
                bias_tiles = {}

                def finish_step(s):
                    g = s // SPC
                    bias_ps = bias_tiles.pop(s)
                    bsb = p1out.tile([H, 2 * L], FP16, tag="bsb",
                                     name=f"bsb_{s}")
                    nc.vector.tensor_scalar_add(bsb[:], bias_ps[:], b2_sb[:, 0:1])
                    j0 = (2 * s) % 8
                    nc.sync.dma_start(
                        shards[g][j0:j0 + 2, :, :].rearrange("k h q -> h k q"),
                        bsb[:].rearrange("h (k q) -> h k q", k=2),
                    )
                    if s % SPC == SPC - 1:
                        nc.gpsimd.collective_compute(
                            "AllGather",
                            mybir.AluOpType.bypass,
                            replica_groups=[list(range(NCORES))],
                            ins=[shards[g][:].opt()],
                            outs=[fulls[g][:].opt()],
                        )

                for s in range(NSTEP):
                    rel2 = p1rel.tile([24, 2 * L], BF16, tag="rel",
                                      name=f"rel_{s}")
                    nc.sync.dma_start(
                        rel2[:], relP_d[:, s * 2 * L:(s + 1) * 2 * L]
                    )
                    bias_ps = p1psb.tile([H, 2 * L], F32, tag="bps",
                                         name=f"bps_{s}")
                    bias_tiles[s] = bias_ps
                    for dc in range(NCH):
                        hid = p1ps.tile([128, 2 * L], F32, tag="hid",
                                        name=f"hid_{s}_{dc}")
                        for j in range(2):
                            nc.tensor.matmul(
                                hid[:, j * L:(j + 1) * L],
                                W1p_sb[:, dc * 128:(dc + 1) * 128],
                                rel2[:, j * L:(j + 1) * L],
                                start=True,
                                stop=True,
                            )
                        gelw = p1gel.tile([128, 2 * L], FP16, tag="gel",
                                          name=f"gel_{s}_{dc}")
                        nc.scalar.activation(
                            gelw[:], hid[:], AF.Gelu, bias=b1_sb[:, dc:dc + 1]
                        )

                        def mm2(s=s, dc=dc, bias_ps=bias_ps, gelw=gelw):
                            for j in range(2):
                                nc.tensor.matmul(
                                    bias_ps[:, j * L:(j + 1) * L],
                                    W2P_sb[:, dc, :],
                                    gelw[:, j * L:(j + 1) * L],
                                    start=(dc == 0),
                                    stop=(dc == NCH - 1),
                                )
                            if dc == NCH - 1:
                                finish_step(s)
                        pend.append(mm2)
                        if len(pend) > 2:
                            pend.popleft()()
                        # one projection micro-op per dc slot as PE filler
                        if s >= 4 and not micro_done:
                            try:
                                next(micro)()
                            except StopIteration:
                                micro_done = True
                while pend:
                    pend.popleft()()
                for op in micro:
                    op()

            # ---- phase 3: transposed attention, 2-stage head pipeline ----
            with (
                tc.tile_pool(name="p3b", bufs=16) as p3b,
                tc.tile_pool(name="p3e", bufs=8) as p3e,
                tc.tile_pool(name="p3r", bufs=2) as p3r,
                tc.tile_pool(name="p3o", bufs=2) as p3o,
                tc.tile_pool(name="lps", bufs=5, space="PSUM") as lps,
                tc.tile_pool(name="avps", bufs=2, space="PSUM") as avps,
                tc.tile_pool(name="rbps", bufs=1, space="PSUM") as rbps,
            ):
                bts = {}

                def load_bias(h):
                    for kc in range(4):
                        bt = p3b.tile([128, L], FP16, tag="bt",
                                      name=f"bt_{h}_{kc}")
                        nc.sync.dma_start(
                            bt[0:64, :],
                            fulls[2 * kc][:, :, h, :].rearrange(
                                "c j q -> (c j) q"
                            ),
                        )
                        nc.sync.dma_start(
                            bt[64:128, :],
                            fulls[2 * kc + 1][:, :, h, :].rearrange(
                                "c j q -> (c j) q"
                            ),
                        )
                        bts[(h, kc)] = bt

                def qkb(h, kc):
                    po = (h % 2) * DH
                    ch = h // 2
                    hs = slice(po, po + DH)
                    ps_l = lps.tile([128, L], F32, tag="lg",
                                    name=f"pl_{h}_{kc}")
                    nc.tensor.matmul(
                        ps_l[:],
                        kT_sb[hs, ch, kc * 128:(kc + 1) * 128],
                        qT_sb[hs, ch, :],
                        start=True,
                        stop=False,
                    )
                    nc.tensor.matmul(
                        ps_l[:],
                        ident[:],
                        bts.pop((h, kc))[:],
                        start=False,
                        stop=True,
                    )
                    exp_t = p3e.tile([128, L], BF16, tag="exp",
                                     name=f"ex_{h}_{kc}")
                    nc.scalar.activation(exp_t[:], ps_l[:], AF.Exp)
                    if dbg and h == 0 and kc == 0:
                        nc.sync.dma_start(dbg_exp[:, :], exp_t[:])
                    return exp_t

                exps = {}

                def stage1(h):
                    if h + 2 < H:
                        load_bias(h + 2)
                    exps[h] = [qkb(h, kc) for kc in range(3)]

                def stage2(h):
                    av = avps.tile([DH + 1, L], F32, tag="av", name=f"av_{h}")
                    e = exps.pop(h)
                    nc.tensor.matmul(
                        av[:], v_sb[:, 0, h, :], e[0][:],
                        start=True, stop=False,
                    )
                    e3 = qkb(h, 3)
                    for kc in (1, 2):
                        nc.tensor.matmul(
                            av[:], v_sb[:, kc, h, :], e[kc][:],
                            start=False, stop=False,
                        )
                    nc.tensor.matmul(
                        av[:], v_sb[:, 3, h, :], e3[:],
                        start=False, stop=True,
                    )
                    rc = p3r.tile([1, L], BF16, tag="rc", name=f"rc_{h}")
                    with nc.allow_low_precision("bf16 softmax scale bcast"):
                        nc.vector.reciprocal(rc[:], av[DH:DH + 1, :])
                    rb = rbps.tile([DH, L], F32, tag="rb", name=f"rb_{h}")
                    nc.tensor.matmul(
                        rb[:], ones1[:], rc[:], start=True, stop=True,
                    )
                    rb_sb = p3r.tile([DH, L], F32, tag="rbs", name=f"rbs_{h}")
                    nc.vector.tensor_copy(rb_sb[:], rb[:])
                    nc.vector.tensor_tensor(
                        attnT[:, h, :], av[0:DH, :], rb_sb[:], op=MULT
                    )

                load_bias(0)
                load_bias(1)
                stage1(0)
                for h in range(H):
                    if h + 1 < H:
                        stage1(h + 1)
                    stage2(h)

                if dbg:
                    nc.sync.dma_start(dbg_full[:], fulls[0][:])
                    nc.sync.dma_start(dbg_qt[:], qT_sb[:])
                    nc.sync.dma_start(dbg_att[:], attnT[:])

                # ---- output projection ----
                for tc4 in range(4):
                    out_sb = p3o.tile([128, D], F32, tag="osb",
                                      name=f"osb_{tc4}")
                    for hf in range(2):
                        ps_o = lps.tile([128, L], F32, tag="lg",
                                        name=f"pso_{tc4}_{hf}")
                        sl = slice(hf * 384, (hf + 1) * 384)
                        for h2 in range(H):
                            nc.tensor.matmul(
                                ps_o[:, 0:384],
                                attnT[:, h2, tc4 * 128:(tc4 + 1) * 128],
                                Wo_sb[:, h2, sl],
                                start=(h2 == 0),
                                stop=(h2 == H - 1),
                            )
                        nc.vector.tensor_tensor(
                            out_sb[:, sl], ps_o[:, 0:384], bo_sb[:, sl], op=ADD
                        )
                    nc.sync.dma_start(
                        out_d[tc4 * 128:(tc4 + 1) * 128, :], out_sb[:]
                    )

    nc.compile()
    return nc


def _get_nc():
    if "nc" not in _CACHE:
        _CACHE["nc"] = _build()
    return _CACHE["nc"]


def _hi_lo(a, dt):
    hi = a.astype(dt)
    lo = (a - hi.astype(np.float32)).astype(dt)
    return hi, lo


def _owned_keys(c):
    # core c owns keys {g*64 + c*8 + j}, ordered by (g, j)
    ks = []
    for g in range(NCHUNK):
        for j in range(8):
            ks.append(g * 64 + c * 8 + j)
    return np.array(ks, dtype=np.int64)


def kernel(
    query,
    key_value,
    query_coords,
    key_coords,
    Wq,
    bq,
    Wk,
    bk,
    Wv,
    bv,
    Wo,
    bo,
    W1,
    b1,
    W2,
    b2,
):
    import ml_dtypes

    query = np.asarray(query, np.float32)
    key_value = np.asarray(key_value, np.float32)
    query_coords = np.asarray(query_coords, np.float32)
    key_coords = np.asarray(key_coords, np.float32)

    def chunked(w, dt=np.float16):  # [768, X] -> [128, 6, X]
        w = np.asarray(w, np.float32).astype(dt)
        return np.ascontiguousarray(w.reshape(NCH, 128, -1).transpose(1, 0, 2))

    def pchunk(b):  # [768] -> [128, 6]
        return np.ascontiguousarray(np.asarray(b, np.float32).reshape(NCH, 128).T)

    WqS = chunked(np.asarray(Wq, np.float32) * np.float32(SCALE))
    Wk_l = chunked(Wk)
    Wv_l = chunked(Wv, ml_dtypes.bfloat16)
    Wo_l = np.ascontiguousarray(
        np.asarray(Wo, np.float32).astype(np.float16)
        .reshape(H, DH, D).transpose(1, 0, 2)
    )
    W2P_l = chunked(W2)  # [128, 6, 12] fp16
    W1f = np.asarray(W1, np.float32)
    W1hi, W1lo = _hi_lo(W1f, ml_dtypes.bfloat16)
    W1P = np.zeros((24, D), ml_dtypes.bfloat16)
    W1P[0:6] = W1hi
    W1P[6:12] = W1hi
    W1P[12:18] = W1lo
    W1P[18:24] = W1lo
    bqS = pchunk(np.asarray(bq, np.float32) * np.float32(SCALE))
    bk_l = pchunk(bk)
    b1_l = pchunk(b1)
    b2_l = np.ascontiguousarray(np.asarray(b2, np.float32).reshape(H, 1))
    bv_b = np.ascontiguousarray(np.broadcast_to(np.asarray(bv, np.float32), (128, D)))
    bo_b = np.ascontiguousarray(np.broadcast_to(np.asarray(bo, np.float32), (128, D)))

    in_maps = []
    for c in range(NCORES):
        ks = _owned_keys(c)
        # rel features for (owned k, all q): [64, 512, 6]
        delta = key_coords[ks][:, None, :] * -1.0 + query_coords[None, :, :]
        rel = np.concatenate([delta, np.abs(delta), np.square(delta)], axis=-1)
        relT = rel.reshape(KS * L, 6).T  # [6, 64*512] (k outer, q inner)
        rhi, rlo = _hi_lo(relT, ml_dtypes.bfloat16)
        relP = np.zeros((24, KS * L), ml_dtypes.bfloat16)
        relP[0:6] = rhi
        relP[6:12] = rlo
        relP[12:18] = rhi
        relP[18:24] = rlo
        in_maps.append(
            {
                "xqT": np.ascontiguousarray(query[c].T).astype(np.float16),
                "kvT": np.ascontiguousarray(key_value[c].T).astype(np.float16),
                "kvTB": np.ascontiguousarray(key_value[c].T).astype(
                    ml_dtypes.bfloat16
                ),
                "relP": relP,
                "WqS": WqS,
                "Wk": Wk_l,
                "Wv": Wv_l,
                "Wo": Wo_l,
                "W1P": W1P,
                "W2P": W2P_l,
                "bqS": bqS,
                "bk": bk_l,
                "b1": b1_l,
                "b2": b2_l,
                "bvb": bv_b,
                "bob": bo_b,
                "idn": np.eye(128, dtype=np.float16),
                "one": np.ones((1, 64), dtype=ml_dtypes.bfloat16),
                "oneh": np.ones((128, 4 * H), dtype=ml_dtypes.bfloat16),
            }
        )

    nc = _get_nc()
    res = bass_utils.run_bass_kernel_spmd(nc, in_maps, core_ids=list(range(NCORES)))
    out = np.stack([res.results[c]["out"] for c in range(NCORES)], axis=0)
    return out.astype(np.float32)
